# revision 1
# baseline (speedup 1.0000x reference)
"""Trainium2 Bass kernel for nn_Attention_Module_15152644620833 (v3).

Reference computation (T=4096, B=8, D=1024, H=64, half=2048):
    q   = x[:half] @ Wq + bq            (half, B, H)
    k   = x @ Wk + bk                   (T, B, H)
    val = x @ Wv + bv                   (T, B, H)
    r   = posenc(T, D) @ Wr + br        (T, H)
    scores[b] = q[b] @ (k[b] + r).T + bias[b][None, :]
        where bias[b][m] = sum(u) * k[m,b,:].sum() + sum(v) * r[m,:].sum()
    causal mask on first `half` key positions, softmax over all T keys,
    out = attn @ val                    (half, B, H)

Sharding: data-parallel over batch, one batch per NeuronCore (8 cores).
Each core receives its batch slice pre-transposed (x.T, contiguous).  The
positional-encoding projection r (identical on all cores) is sharded: each
core computes a 512-key slice of r.T and the full r.T is AllGathered.

Per-core device algorithm (f32r matmuls, fp32 PSUM):
    K2 (128, T):  rows 0:64 = k.T + bk, rows 64:128 = r.T + br
    q2 (128, half): rows 0:64 = q.T + bq, rows 64:128 = the same q.T
        -> scoresT(m,t) = K2[:,mtile].T @ q2 = q.k + q.r   (K=128)
    softmax key bias folded multiplicatively into val:
        exp(s + bias[m]) = exp(s)*eb[m]; eb scales both the val columns and
        the ones column (denominator), so attn is unchanged (exact).
        bias[m] = K2[:,m].T @ [u_sum x64; v_sum x64]   (one N=1 matmul/tile)
    causal mask: accumulate identity.T @ maskA (-1e30) into scores PSUM of
        diagonal tiles; fully-masked tiles are skipped.
    expT = exp(scoresT)  (no max subtraction: |scores| < ~60, safe in f32)
    outT (65, 512) += valaug[mtile].T @ expT  per query chunk (col 64 of
        valaug = eb -> row 64 of outT = softmax denominator)
    out (128, 64) = transpose(outT) * (1/denominator)

Schedule: sweep 1 streams x.T and runs all gather-independent projections
while the AllGather is in flight (its DMAs ride the ACT HWDGE ring so they
cannot head-of-line-block the x.T stream on the SP ring); sweep 2 runs
attention query-chunk-outer with the attnval matmuls software-pipelined two
exp-groups behind the score matmuls.
"""

import math

import numpy as np

T, B, D, H = 4096, 8, 1024, 64
HALF = T // 2
P = 128
DC = D // P          # 8 d-chunks
NCH = T // 512       # 8 key chunks of 512
NTQ = HALF // 512    # 4 query chunks of 512
MT = T // P          # 32 key tiles of 128
NCORES = 8

_CACHE = {}


def _posenc_T():
    """Constant positional encoding, transposed to (D, T), float32."""
    pos = np.arange(T, dtype=np.float32)[:, None]
    div = np.exp(
        (np.arange(0, D, 2, dtype=np.float32)
         * np.float32(-(math.log(10000.0) / D))).astype(np.float32)
    ).astype(np.float32)
    ang = (pos * div).astype(np.float32)
    pe = np.stack([np.sin(ang), np.cos(ang)], axis=-1).reshape(T, D)
    return np.ascontiguousarray(pe.astype(np.float32).T)


def _live(tq, mt):
    """Key tile mt contributes to query chunk tq (not fully masked)."""
    m0 = mt * P
    return not (m0 >= tq * 512 + 512 and m0 < HALF)


def _is_diag(tq, mt):
    return tq * 512 <= mt * P < tq * 512 + 512


def _build_module():
    import concourse.bacc as bacc
    import concourse.bass_isa as bass_isa
    import concourse.mybir as mybir
    from concourse.masks import make_identity
    from concourse.tile import TileContext

    f32 = mybir.dt.float32
    f32r = mybir.dt.float32r
    Exp = mybir.ActivationFunctionType.Exp

    nc = bacc.Bacc(num_devices=NCORES)

    xT_h = nc.dram_tensor("xT", [D, T], f32r, kind="ExternalInput")
    peTs_h = nc.dram_tensor("peTs", [D, 512], f32r, kind="ExternalInput")
    wkv_h = nc.dram_tensor("wkv", [D, 2 * H], f32r, kind="ExternalInput")
    wqq_h = nc.dram_tensor("wqq", [D, 2 * H], f32r, kind="ExternalInput")
    wr2_h = nc.dram_tensor("wr2", [D, 2 * H], f32r, kind="ExternalInput")
    bkv_h = nc.dram_tensor("bkv", [2 * H, 1], f32, kind="ExternalInput")
    bqq_h = nc.dram_tensor("bqq", [2 * H, 1], f32, kind="ExternalInput")
    br_h = nc.dram_tensor("br", [H, 1], f32, kind="ExternalInput")
    u_h = nc.dram_tensor("u", [H, 1], f32, kind="ExternalInput")
    v_h = nc.dram_tensor("v", [H, 1], f32, kind="ExternalInput")
    out_h = nc.dram_tensor("out", [HALF, H], f32, kind="ExternalOutput")

    xT_r = xT_h[:, :].rearrange("(c p) t -> p c t", p=P)       # (128, 8, T)
    peTs_r = peTs_h[:, :].rearrange("(c p) t -> p c t", p=P)   # (128, 8, 512)
    wkv_r = wkv_h[:, :].rearrange("(c p) h -> p c h", p=P)
    wqq_r = wqq_h[:, :].rearrange("(c p) h -> p c h", p=P)
    wr2_r = wr2_h[:, :].rearrange("(c p) h -> p c h", p=P)
    out_r = out_h[:, :].rearrange("(g p) h -> p g h", p=P)     # (128, 16, 64)

    with TileContext(nc) as tc, tc.tile_pool(name="persist", bufs=1) as persist:

        def _tile(shape, name, dt=f32):
            return persist.tile(shape, dt, name=name)

        # ---- persistent SBUF tiles -------------------------------------
        wkv_sb = _tile([P, DC, 2 * H], "wkv_sb", f32r)
        wqq_sb = _tile([P, DC, 2 * H], "wqq_sb", f32r)
        wr2_sb = _tile([P, DC, 2 * H], "wr2_sb", f32r)
        bkv_sb = _tile([2 * H, 1], "bkv_sb")
        bqq_sb = _tile([2 * H, 1], "bqq_sb")
        brc_sb = _tile([2 * H, 1], "brc_sb")    # br loaded at rows 64:128
        u_cl = _tile([H, 1], "u_cl")
        v_cl = _tile([H, 1], "v_cl")
        u_all = _tile([H, 1], "u_all")
        v_all = _tile([H, 1], "v_all")
        uvf = _tile([2 * H, 1], "uvf")
        uv_col = _tile([2 * H, 4], "uv_col", f32r)
        id_sb = _tile([P, P], "id_sb", f32r)
        maskA = _tile([P, 4, 512], "maskA", f32r)
        K2 = _tile([P, T], "K2", f32r)          # 0:64 k.T+bk, 64:128 r.T+br
        vT = _tile([P, T], "vT", f32r)          # rows 64:128 = v.T+bv
        q2T = _tile([P, HALF], "q2T", f32r)     # rows 0:64 and 64:128 = q.T
        valaug = _tile([P, MT, H + 1], "valaug", f32r)
        ebias = _tile([P, MT], "ebias")
        outall = _tile([P, HALF // P, H], "outall")

        # ---- constants / small setup -----------------------------------
        nc.sync.dma_start(wr2_sb[:], wr2_r)
        nc.sync.dma_start(wkv_sb[:], wkv_r)
        nc.sync.dma_start(wqq_sb[:], wqq_r)
        nc.sync.dma_start(bkv_sb[:], bkv_h[:, :])
        nc.sync.dma_start(bqq_sb[:], bqq_h[:, :])
        nc.sync.dma_start(brc_sb[H : 2 * H, :], br_h[:, :])
        nc.sync.dma_start(u_cl[:], u_h[:, :])
        nc.sync.dma_start(v_cl[:], v_h[:, :])

        with (
            tc.tile_pool(name="xstream", bufs=2) as xpool,
            tc.tile_pool(name="dramp", bufs=1, space="DRAM") as dramp,
        ):
            # ---- r.T shard + AllGather kickoff (identical r everywhere) -
            with tc.tile_pool(name="ppj", bufs=1, space="PSUM") as ppj:
                pet = xpool.tile([P, DC, 512], f32r, name="pet", tag="xt")
                nc.sync.dma_start(pet[:], peTs_r)
                rp = ppj.tile([P, 512], f32, name="rp", tag="kv", bufs=2)
                for dc in range(DC):
                    nc.tensor.matmul(
                        rp[:], wr2_sb[:, dc, :], pet[:, dc, :],
                        start=(dc == 0), stop=(dc == DC - 1),
                    )
                rloc_sb = xpool.tile([P, 512], f32r, name="rloc_sb", tag="rloc")
                nc.vector.tensor_scalar_add(
                    rloc_sb[H:P, :], rp[H:P, :], brc_sb[H : 2 * H, :]
                )
                rloc_dr = dramp.tile([H, 512], f32r, name="rloc_dr")
                nc.scalar.dma_start(rloc_dr[:], rloc_sb[H:P, :])
                rTg_dr = dramp.tile([NCORES * H, 512], f32r, name="rTg_dr",
                                    addr_space="Shared")
                nc.gpsimd.collective_compute(
                    "AllGather", mybir.AluOpType.bypass,
                    replica_groups=[list(range(NCORES))],
                    ins=[rloc_dr[:]], outs=[rTg_dr[:]],
                )
                # on the ACT HWDGE ring: this DMA waits on the collective,
                # and on the SP ring it would head-of-line-block the x.T
                # chunk stream behind it
                nc.scalar.dma_start(
                    K2[H:P, :].rearrange("h (c m) -> h c m", c=NCH),
                    rTg_dr[:].rearrange("(c h) m -> h c m", h=H),
                )

                # f32r tiles cannot be produced by memset/affine_select directly
                # (ISA/verifier); build constants in f32 scratch, cast-copy on DVE.
                with tc.tile_pool(name="setupf", bufs=1) as setupf:
                    idf = setupf.tile([P, P], f32, name="idf")
                    make_identity(nc, idf[:])
                    nc.vector.tensor_copy(id_sb[:], idf[:])
                    maskAf = setupf.tile([P, 4, 512], f32, name="maskAf")
                    nc.gpsimd.memset(maskAf[:], 0.0)
                    for rel in range(4):
                        nc.gpsimd.affine_select(
                            out=maskAf[:, rel, :], in_=maskAf[:, rel, :],
                            compare_op=mybir.AluOpType.is_ge, fill=-1e30,
                            base=-P * rel, pattern=[[1, 512]], channel_multiplier=-1,
                        )
                    nc.vector.tensor_copy(maskA[:], maskAf[:])
                    onesf = setupf.tile([P, MT], f32, name="onesf")
                    nc.gpsimd.memset(onesf[:], 1.0)
                    nc.vector.tensor_copy(valaug[:, :, H], onesf[:, 0:MT])

                # sum(u) broadcast to rows 0:64, sum(v) to rows 64:128 of uv_col
                nc.gpsimd.partition_all_reduce(u_all[:], u_cl[:], H, bass_isa.ReduceOp.add)
                nc.gpsimd.partition_all_reduce(v_all[:], v_cl[:], H, bass_isa.ReduceOp.add)
                nc.vector.tensor_copy(uvf[0:H, :], u_all[:])
                nc.sync.dma_start(uvf[H : 2 * H, :], v_all[:])  # partition shift
                nc.vector.tensor_copy(uv_col[:], uvf[:, 0:1].to_broadcast((2 * H, 4)))


                # ---- sweep 1: gather-independent projections ------------
                for c in range(NCH):
                    sl = slice(c * 512, (c + 1) * 512)
                    xt = xpool.tile([P, DC, 512], f32r, name="xt", tag="xt")
                    nc.sync.dma_start(xt[:], xT_r[:, :, sl])

                    kvp = ppj.tile([P, 512], f32, name="kvp", tag="kv", bufs=2)
                    for dc in range(DC):
                        nc.tensor.matmul(
                            kvp[:], wkv_sb[:, dc, :], xt[:, dc, :],
                            start=(dc == 0), stop=(dc == DC - 1),
                        )
                    nc.vector.tensor_scalar_add(
                        K2[0:H, sl], kvp[0:H, :], bkv_sb[0:H, :]
                    )
                    nc.vector.tensor_scalar_add(
                        vT[H:P, sl], kvp[H:P, :], bkv_sb[H : 2 * H, :]
                    )

                    if c < NTQ:
                        qp = ppj.tile([P, 512], f32, name="qp", tag="kv", bufs=2)
                        for dc in range(DC):
                            nc.tensor.matmul(
                                qp[:], wqq_sb[:, dc, :], xt[:, dc, :],
                                start=(dc == 0), stop=(dc == DC - 1),
                            )
                        nc.vector.tensor_scalar_add(q2T[:, sl], qp[:], bqq_sb[:])

                    for j in range(4):
                        mt = c * 4 + j
                        msl = slice(mt * P, (mt + 1) * P)
                        vp = ppj.tile([P, 512], f32r, name="vp", tag="kv",
                                      bufs=2)[:, 0:H]
                        nc.tensor.transpose(vp[:], vT[H:P, msl], id_sb[H:P, H:P])
                        nc.vector.tensor_copy(valaug[:, mt, 0:H], vp[:])

                # ---- key bias -> eb, folded into valaug (gather-gated) --
                for c in range(NCH):
                    bp = ppj.tile([P, 512], f32, name="bp", tag="kv",
                                  bufs=2)[:, 0:16]
                    for j in range(4):
                        mt = c * 4 + j
                        msl = slice(mt * P, (mt + 1) * P)
                        nc.tensor.matmul(
                            bp[:, 4 * j : 4 * j + 4], K2[:, msl], uv_col[:],
                            start=True, stop=True,
                        )
                    nc.scalar.activation(
                        ebias[:, c * 4 : (c + 1) * 4], bp[:, 0:16:4], Exp
                    )
                    for j in range(4):
                        mt = c * 4 + j
                        nc.vector.tensor_scalar_mul(
                            valaug[:, mt, :], valaug[:, mt, :],
                            ebias[:, mt : mt + 1],
                        )

            # ---- sweep 2: attention, query-chunk outer ------------------
            with (
                tc.tile_pool(name="expp", bufs=4) as exp_pool,
                tc.tile_pool(name="posb", bufs=2) as osb_pool,
                tc.tile_pool(name="pinv", bufs=2) as inv_pool,
                tc.tile_pool(name="ps_s", bufs=2, space="PSUM") as pp_s,
                tc.tile_pool(name="ps_o", bufs=2, space="PSUM") as pp_o,
            ):
                for tq in range(NTQ):
                    tsl = slice(tq * 512, (tq + 1) * 512)
                    mts = [mt for mt in range(MT) if _live(tq, mt)]
                    groups = [mts[i : i + 3] for i in range(0, len(mts), 3)]
                    oT_ps = pp_o.tile([H + 1, 512], f32, name="oT_ps")
                    n_done = 0
                    pend = []
                    for g in groups + [None, None]:
                        if g is not None:
                            sp = pp_s.tile([P, 3, 512], f32, name="sp", tag="sp")
                            for i, mt in enumerate(g):
                                msl = slice(mt * P, (mt + 1) * P)
                                diag = _is_diag(tq, mt)
                                nc.tensor.matmul(
                                    sp[:, i, :], K2[:, msl], q2T[:, tsl],
                                    start=True, stop=not diag,
                                )
                                if diag:
                                    nc.tensor.matmul(
                                        sp[:, i, :], id_sb[:, :],
                                        maskA[:, mt - tq * 4, :],
                                        start=False, stop=True,
                                    )
                            ex = exp_pool.tile([P, 3, 512], f32r, name="ex")
                            nc.scalar.activation(
                                ex[:, 0 : len(g), :], sp[:, 0 : len(g), :], Exp
                            )
                        # attnval emitted two groups late, so the next two
                        # groups' score matmuls sit ahead of it in the
                        # in-order PE queue and PE never stalls on exp
                        if g is not None:
                            pend.append((g, ex))
                        if (len(pend) > 2) or (g is None and pend):
                            pg, pex = pend.pop(0)
                            for i, mt in enumerate(pg):
                                nc.tensor.matmul(
                                    oT_ps[:], valaug[:, mt, :], pex[:, i, :],
                                    start=(n_done == 0),
                                    stop=(n_done == len(mts) - 1),
                                )
                                n_done += 1
                    oT_sb = osb_pool.tile([H + 1, 512], f32, name="oT_sb")
                    nc.vector.tensor_copy(oT_sb[:], oT_ps[:])
                    for j in range(4):
                        # share the accumulator slots (free once oT_sb is
                        # copied) instead of the score slots, which would
                        # stall the next query chunk's score matmuls
                        tp = pp_o.tile([P, H + 1], f32, name="tp", tag="oT_ps")
                        nc.tensor.transpose(
                            tp[:], oT_sb[:, j * P : (j + 1) * P],
                            id_sb[0 : H + 1, 0 : H + 1].bitcast(f32),
                        )
                        inv = inv_pool.tile([P, 1], f32, name="inv")
                        nc.vector.reciprocal(inv[:], tp[:, H : H + 1])
                        nc.vector.tensor_scalar_mul(
                            outall[:, tq * 4 + j, :], tp[:, 0:H], inv[:]
                        )
                nc.sync.dma_start(out_r, outall[:])

    nc.compile()
    return nc


def _get_module():
    if "nc" not in _CACHE:
        _CACHE["nc"] = _build_module()
    return _CACHE["nc"]


def _make_in_maps(inputs):
    inp = np.asarray(inputs["inp_data"], dtype=np.float32)
    Wq = np.asarray(inputs["Wq"], dtype=np.float32)
    bq = np.asarray(inputs["bq"], dtype=np.float32)
    Wk = np.asarray(inputs["Wk"], dtype=np.float32)
    bk = np.asarray(inputs["bk"], dtype=np.float32)
    Wv = np.asarray(inputs["Wv"], dtype=np.float32)
    bv = np.asarray(inputs["bv"], dtype=np.float32)
    Wr = np.asarray(inputs["Wr"], dtype=np.float32)
    br = np.asarray(inputs["br"], dtype=np.float32)
    u = np.asarray(inputs["u"], dtype=np.float32)
    v = np.asarray(inputs["v"], dtype=np.float32)

    if "peT" not in _CACHE:
        _CACHE["peT"] = _posenc_T()
    peT = _CACHE["peT"]
    common = {
        "wkv": np.ascontiguousarray(np.concatenate([Wk, Wv], axis=1)),
        "wqq": np.ascontiguousarray(np.concatenate([Wq, Wq], axis=1)),
        "wr2": np.ascontiguousarray(
            np.concatenate([np.zeros_like(Wr), Wr], axis=1)
        ),
        "bkv": np.ascontiguousarray(np.concatenate([bk, bv]).reshape(2 * H, 1)),
        "bqq": np.ascontiguousarray(np.concatenate([bq, bq]).reshape(2 * H, 1)),
        "br": np.ascontiguousarray(br.reshape(H, 1)),
        "u": np.ascontiguousarray(u.reshape(H, 1)),
        "v": np.ascontiguousarray(v.reshape(H, 1)),
    }
    in_maps = []
    for b in range(NCORES):
        m = {
            "xT": np.ascontiguousarray(inp[:, b, :].T),
            "peTs": np.ascontiguousarray(peT[:, b * 512 : (b + 1) * 512]),
        }
        m.update(common)
        in_maps.append(m)
    return in_maps


def _run(in_maps, trace=False):
    from concourse.bass_utils import run_bass_kernel_spmd

    nc = _get_module()
    return run_bass_kernel_spmd(
        nc, in_maps, core_ids=list(range(NCORES)), trace=trace
    )


def _timed_run(in_maps, iters=5, reps=1):
    """Replicates bass2jax.run_bass_via_pjrt's multi-core path, but keeps the
    jitted callable and device-resident inputs so repeated executions can be
    wall-clock timed (no NTFF profiling is available through the axon client).
    """
    import time

    import jax
    import concourse.mybir as mybir
    from concourse.bass2jax import (
        _bass_exec_p,
        install_neuronx_cc_hook,
        partition_id_tensor,
    )
    from jax.experimental.shard_map import shard_map
    from jax.sharding import Mesh, NamedSharding, PartitionSpec

    nc = _get_module()
    install_neuronx_cc_hook()
    partition_name = nc.partition_id_tensor.name if nc.partition_id_tensor else None

    in_names, out_names, out_avals, zero_shapes = [], [], [], []
    for alloc in nc.m.functions[0].allocations:
        if not isinstance(alloc, mybir.MemoryLocationSet):
            continue
        name = alloc.memorylocations[0].name
        if alloc.kind == "ExternalInput":
            if name != partition_name:
                in_names.append(name)
        elif alloc.kind == "ExternalOutput":
            out_names.append(name)
            shape = tuple(alloc.tensor_shape)
            dtype = mybir.dt.np(alloc.dtype)
            out_avals.append(jax.core.ShapedArray(shape, dtype))
            zero_shapes.append((shape, dtype))
    n_params = len(in_names)
    all_names = in_names + out_names
    if partition_name is not None:
        all_names = all_names + [partition_name]
    donate = tuple(range(n_params, n_params + len(out_names)))

    def _body(*args):
        operands = list(args)
        if partition_name is not None:
            operands.append(partition_id_tensor())
        outs = _bass_exec_p.bind(
            *operands,
            out_avals=tuple(out_avals),
            in_names=tuple(all_names),
            out_names=tuple(out_names),
            lowering_input_output_aliases=(),
            sim_require_finite=True,
            sim_require_nnan=True,
            nc=nc,
        )
        return tuple(outs)

    devices = jax.devices()[:NCORES]
    mesh = Mesh(np.asarray(devices), ("core",))
    spec = PartitionSpec("core")
    in_specs = (spec,) * (n_params + len(out_names))
    sharded = jax.jit(
        shard_map(
            _body, mesh=mesh, in_specs=in_specs,
            out_specs=(spec,) * len(out_names), check_rep=False,
        ),
        donate_argnums=donate,
        keep_unused=True,
    )
    sharding = NamedSharding(mesh, spec)
    concat_in = [
        jax.device_put(
            np.concatenate([in_maps[c][nm] for c in range(NCORES)], axis=0), sharding
        )
        for nm in in_names
    ]

    def zeros():
        return [
            jax.device_put(np.zeros((NCORES * s[0], *s[1:]), d), sharding)
            for (s, d) in zero_shapes
        ]

    out = sharded(*concat_in, *zeros())
    jax.block_until_ready(out)
    times = []
    for _ in range(iters):
        zs = zeros()
        jax.block_until_ready(zs)
        t0 = time.perf_counter()
        out = sharded(*concat_in, *zs)
        jax.block_until_ready(out)
        times.append(time.perf_counter() - t0)
    results = {
        nm: np.asarray(out[i]).reshape(NCORES, *out_avals[i].shape)
        for i, nm in enumerate(out_names)
    }
    return results, times


def kernel(**inputs) -> np.ndarray:
    in_maps = _make_in_maps(inputs)
    res = _run(in_maps, trace=False)
    out = np.stack([res.results[b]["out"] for b in range(NCORES)], axis=1)
    return np.ascontiguousarray(out.astype(np.float32))



# revision 40
# speedup vs baseline: 1.4949x; 1.4949x over previous
"""Trainium2 Bass kernel for nn_Attention_Module_15152644620833 (v7).

Reference computation (T=4096, B=8, D=1024, H=64, half=2048):
    q   = x[:half] @ Wq + bq            (half, B, H)
    k   = x @ Wk + bk                   (T, B, H)
    val = x @ Wv + bv                   (T, B, H)
    r   = posenc(T, D) @ Wr + br        (T, H)
    scores[b] = q[b] @ (k[b] + r).T + bias[b][None, :]
        where bias[b][m] = sum(u) * k[m,b,:].sum() + sum(v) * r[m,:].sum()
    causal mask on first `half` key positions, softmax over all T keys,
    out = attn @ val                    (half, B, H)

Sharding: data-parallel over batch, one batch per NeuronCore (8 cores).

v7 design (cost-model driven):
  - fp16 stream/weights/K2/q2T (half the HBM bytes of f32 at full PE rate
    and ~f32r precision: 10-bit mantissa, exact f32 PSUM accumulate);
    bf16 for exp outputs / valaug (fp16 range too small for exp(~60)).
  - No collective: the modeled AllGather costs 15us fixed + transfer plus
    ~10us of latency dribble, all on the critical path to every score
    matmul. Instead each core streams the full posenc.T (8.4MB fp16) in
    512-column chunks paired with the x.T chunks and projects r locally
    (+13.7us PE, overlapped under the DMA-bound stream).
  - Chunk pairs stream in order [0,4,5,6,7,1,2,3]: after the first pair,
    query chunk 0's diagonal tiles are already scoreable, so the attention
    pipeline (and the ACT-bound exp stream) starts at ~10us instead of
    ~40us. Per-chunk: k/v/q/r projections, bias->eb fold, v transpose.
  - One flat attention pipeline across all query chunks, jobs ordered by
    data arrival; scores/exp run 3 groups ahead of attnval.
  - Causal mask: accumulate identity.T @ maskA (-30000, fp16-exact) into
    the scores PSUM of diagonal tiles.
  - PSUM: proj ring 2 banks, score ring 2x2 banks, oT/transpose ring 2.

Per-core algorithm (fp16 matmuls, fp32 PSUM):
    K2 (128, T):  rows 0:64 = k.T + bk, rows 64:128 = r.T + br
    q2 (128, half): rows 0:64 = rows 64:128 = q.T + bq
        -> scoresT(m,t) = K2[:,mtile].T @ q2 = q.k + q.r   (K=128)
    softmax key bias folded multiplicatively into val:
        exp(s + bias[m]) = exp(s)*eb[m]; eb scales both the val columns and
        the ones column (denominator), so attn is unchanged (exact).
        bias[m] = K2[:,m].T @ [u_sum x64; v_sum x64]   (one N=4 matmul/tile)
    expT = exp(scoresT) in bf16 (no max subtraction: |scores| < ~60)
    outT (65, 512) += valaug[mtile].T @ expT  per query chunk (col 64 of
        valaug = eb -> row 64 of outT = softmax denominator)
    out (128, 64) = transpose(outT) * (1/denominator)
"""

import math

import numpy as np

T, B, D, H = 4096, 8, 1024, 64
HALF = T // 2
P = 128
DC = D // P          # 8 d-chunks
NCH = T // 512       # 8 key chunks of 512
NTQ = HALF // 512    # 4 query chunks of 512
MT = T // P          # 32 key tiles of 128
NCORES = 8
CHUNK_ORDER = [0, 4, 5, 6, 7, 1, 2, 3]
MASK_NEG = -30000.0  # fp16-exact, exp(s + MASK_NEG) == 0 for |s| < 1e4
GROUP = 2            # score tiles per exp group (2 PSUM banks each)
GCH = [1, 2, 3]      # chunks whose r rows come from the AllGather (needed
                     # latest: key chunks 1-3 only feed tq1-3 scores/bias)
GSH = 512 * 3 // NCORES   # 192 posenc columns projected per core
PIPE = 12             # attnval groups of pipeline lag behind scores/exp

_CACHE = {}


def _posenc_T():
    """Constant positional encoding, transposed to (D, T), float32."""
    pos = np.arange(T, dtype=np.float32)[:, None]
    div = np.exp(
        (np.arange(0, D, 2, dtype=np.float32)
         * np.float32(-(math.log(10000.0) / D))).astype(np.float32)
    ).astype(np.float32)
    ang = (pos * div).astype(np.float32)
    pe = np.stack([np.sin(ang), np.cos(ang)], axis=-1).reshape(T, D)
    return np.ascontiguousarray(pe.astype(np.float32).T)


def _live(tq, mt):
    """Key tile mt contributes to query chunk tq (not fully masked)."""
    m0 = mt * P
    return not (m0 >= tq * 512 + 512 and m0 < HALF)


def _is_diag(tq, mt):
    return tq * 512 <= mt * P < tq * 512 + 512


def _build_module():
    import concourse.bacc as bacc
    import concourse.bass_isa as bass_isa
    import concourse.mybir as mybir
    from concourse.masks import make_identity
    from concourse.tile import TileContext

    f32 = mybir.dt.float32
    f16 = mybir.dt.float16
    bf16 = mybir.dt.bfloat16
    Exp = mybir.ActivationFunctionType.Exp

    nc = bacc.Bacc(num_devices=NCORES)

    xT_h = nc.dram_tensor("xT", [D, T], f16, kind="ExternalInput")
    peT_h = nc.dram_tensor("peT", [D, T], f16, kind="ExternalInput")
    peS_h = nc.dram_tensor("peS", [P, DC * GSH], f16, kind="ExternalInput")
    wkv_h = nc.dram_tensor("wkv", [P, DC * 2 * H], f16, kind="ExternalInput")
    wqq_h = nc.dram_tensor("wqq", [P, DC * 2 * H], f16, kind="ExternalInput")
    wr2_h = nc.dram_tensor("wr2", [P, DC * 2 * H], f16, kind="ExternalInput")
    bkv_h = nc.dram_tensor("bkv", [2 * H, 1], f32, kind="ExternalInput")
    bqq_h = nc.dram_tensor("bqq", [2 * H, 1], f32, kind="ExternalInput")
    br_h = nc.dram_tensor("br", [H, 1], f32, kind="ExternalInput")
    u_h = nc.dram_tensor("u", [H, 1], f32, kind="ExternalInput")
    v_h = nc.dram_tensor("v", [H, 1], f32, kind="ExternalInput")
    out_h = nc.dram_tensor("out", [HALF, H], f32, kind="ExternalOutput")

    xT_r = xT_h[:, :].rearrange("(c p) t -> p c t", p=P)       # (128, 8, T)
    peT_r = peT_h[:, :].rearrange("(c p) t -> p c t", p=P)     # (128, 8, T)
    # packed host-side as direct SBUF images: per-partition rows are one
    # contiguous 2KB+ run, so the DMA descriptors stay above the 512B
    # full-bandwidth threshold (the [D, width] layout's 256B rows pay 2x)
    peS_r = peS_h[:, :].rearrange("p (c t) -> p c t", c=DC)
    wkv_r = wkv_h[:, :].rearrange("p (c h) -> p c h", c=DC)
    wqq_r = wqq_h[:, :].rearrange("p (c h) -> p c h", c=DC)
    wr2_r = wr2_h[:, :].rearrange("p (c h) -> p c h", c=DC)
    out_r = out_h[:, :].rearrange("(g p) h -> p g h", p=P)     # (128, 16, 64)

    with TileContext(nc) as tc, tc.tile_pool(name="persist", bufs=1) as persist:

        def _tile(shape, name, dt=f32):
            return persist.tile(shape, dt, name=name)

        # ---- persistent SBUF tiles -------------------------------------
        wkv_sb = _tile([P, DC, 2 * H], "wkv_sb", f16)
        wqq_sb = _tile([P, DC, 2 * H], "wqq_sb", f16)
        wr2_sb = _tile([P, DC, 2 * H], "wr2_sb", f16)
        bkv_sb = _tile([2 * H, 1], "bkv_sb")
        bqq_sb = _tile([2 * H, 1], "bqq_sb")
        brc_sb = _tile([2 * H, 1], "brc_sb")    # br loaded at rows 64:128
        u_cl = _tile([H, 1], "u_cl")
        v_cl = _tile([H, 1], "v_cl")
        u_all = _tile([H, 1], "u_all")
        v_all = _tile([H, 1], "v_all")
        uvf = _tile([2 * H, 1], "uvf")
        uv_col = _tile([2 * H, 4], "uv_col", f16)
        idf = _tile([P, P], "idf")              # f32 identity (transposes)
        peS_sb = _tile([P, DC, GSH], "peS_sb", f16)
        id_sb = _tile([P, P], "id_sb", f16)
        maskA = _tile([P, 4, 512], "maskA", f16)
        K2 = _tile([P, T], "K2", f16)           # 0:64 k.T+bk, 64:128 r.T+br
        q2T = _tile([P, HALF], "q2T", f16)      # rows 0:64 and 64:128 = q.T
        valaug = _tile([P, MT, H + 1], "valaug", bf16)
        ebias = _tile([P, MT], "ebias")
        outall = _tile([P, HALF // P, H], "outall")

        with (
            tc.tile_pool(name="xstream", bufs=2) as xpool,
            tc.tile_pool(name="pestream", bufs=2) as pepool,
            tc.tile_pool(name="vtmpp", bufs=2) as vtmp_pool,
            tc.tile_pool(name="dramp", bufs=1, space="DRAM") as dramp,
            tc.tile_pool(name="ppj", bufs=1, space="PSUM") as ppj,
            tc.tile_pool(name="spp", bufs=1, space="PSUM") as spp,
            tc.tile_pool(name="acc", bufs=1, space="PSUM") as accp,
            tc.tile_pool(name="expp", bufs=14) as exp_pool,
            tc.tile_pool(name="posb", bufs=2) as osb_pool,
            tc.tile_pool(name="pinv", bufs=2) as inv_pool,
        ):
            # ---- constants (identity needed by first v transpose) ------
            # fp16 tiles can't be produced by memset/affine_select directly;
            # build in f32 scratch, cast-copy on DVE.
            make_identity(nc, idf[:])
            nc.vector.tensor_copy(id_sb[:], idf[:])
            with tc.tile_pool(name="setupf", bufs=1) as setupf:
                maskAf = setupf.tile([P, 4, 512], f32, name="maskAf")
                nc.gpsimd.memset(maskAf[:], 0.0)
                for rel in range(4):
                    nc.gpsimd.affine_select(
                        out=maskAf[:, rel, :], in_=maskAf[:, rel, :],
                        compare_op=mybir.AluOpType.is_ge, fill=MASK_NEG,
                        base=-P * rel, pattern=[[1, 512]], channel_multiplier=-1,
                    )
                nc.vector.tensor_copy(maskA[:], maskAf[:])
                onesf = setupf.tile([P, MT], f32, name="onesf")
                nc.gpsimd.memset(onesf[:], 1.0)
                nc.vector.tensor_copy(valaug[:, :, H], onesf[:, 0:MT])

            # ---- first-chunk DMAs, interleaved with weights, so the first
            # projection starts at ~6.5us instead of after every weight ----
            nc.sync.dma_start(wkv_sb[:], wkv_r)
            nc.sync.dma_start(bkv_sb[:], bkv_h[:, :])
            c0 = CHUNK_ORDER[0]
            xt0 = xpool.tile([P, DC, 512], f16, name="xt", tag="xt")
            nc.sync.dma_start(xt0[:], xT_r[:, :, c0 * 512 : (c0 + 1) * 512])
            nc.sync.dma_start(wr2_sb[:], wr2_r)
            nc.sync.dma_start(peS_sb[:], peS_r)
            nc.sync.dma_start(wqq_sb[:], wqq_r)
            pet0 = pepool.tile([P, DC, 512], f16, name="pet", tag="pet")
            nc.sync.dma_start(pet0[:], peT_r[:, :, c0 * 512 : (c0 + 1) * 512])
            nc.sync.dma_start(bqq_sb[:], bqq_h[:, :])
            nc.sync.dma_start(brc_sb[H : 2 * H, :], br_h[:, :])
            nc.sync.dma_start(u_cl[:], u_h[:, :])
            nc.sync.dma_start(v_cl[:], v_h[:, :])
            nc.gpsimd.partition_all_reduce(u_all[:], u_cl[:], H, bass_isa.ReduceOp.add)
            nc.gpsimd.partition_all_reduce(v_all[:], v_cl[:], H, bass_isa.ReduceOp.add)
            nc.vector.tensor_copy(uvf[0:H, :], u_all[:])
            nc.sync.dma_start(uvf[H : 2 * H, :], v_all[:])  # partition shift
            nc.vector.tensor_copy(uv_col[:], uvf[:, 0:1].to_broadcast((2 * H, 4)))

            # ---- PE warm-up: the p-state model runs matmuls at half/quarter
            # clock for ~3us after any idle period, so keep PE busy on
            # throwaway identity products until the first chunk lands ------
            scratch0 = ppj.tile([P, P], f32, name="scratch0", tag="proj",
                                bufs=2)
            for _ in range(16):
                nc.tensor.matmul(
                    scratch0[:], idf[:, :], idf[:, :],
                    start=True, stop=True,
                )

            # ---- attention job list, grouped by earliest-ready pair -----
            arr = {c: i for i, c in enumerate(CHUNK_ORDER)}
            jobs_at = {pi: [] for pi in range(NCH)}
            ntiles = {}
            for tq in range(NTQ):
                mts = [mt for mt in range(MT) if _live(tq, mt)]
                # diagonal tiles are triangular: only columns >= 128*rel are
                # live, so scores/mask/attnval run partial-width on them. The
                # first and last attnval of each query chunk must be
                # full-width (PSUM start/stop must cover every column), so
                # when the diagonal chunk arrives last its tiles are fed in
                # descending rel order (ending on the full-width rel=0).
                first_arr = min(arr[mt // 4] for mt in mts)
                mts.sort(key=lambda mt: (
                    arr[mt // 4],
                    -mt if (_is_diag(tq, mt) and arr[mt // 4] != first_arr)
                    else mt,
                ))
                ntiles[tq] = len(mts)
                for i in range(0, len(mts), GROUP):
                    g = mts[i : i + GROUP]
                    ready = max(arr[tq], max(arr[mt // 4] for mt in g))
                    jobs_at[ready].append((tq, g))

            oT_ps = {}
            n_done = {tq: 0 for tq in range(NTQ)}
            pend = []

            def _epilogue(tq):
                oT_sb = osb_pool.tile([H + 1, 512], f32, name="oT_sb")
                for j in range(4):
                    nc.vector.tensor_copy(
                        oT_sb[:, j * P : (j + 1) * P],
                        oT_ps[tq][:, j * P : (j + 1) * P],
                    )
                    tp = accp.tile([P, H + 1], f32, name="tp", tag="acc",
                                   bufs=2)
                    nc.tensor.transpose(
                        tp[:], oT_sb[:, j * P : (j + 1) * P],
                        idf[0 : H + 1, 0 : H + 1],
                    )
                    inv = inv_pool.tile([P, 1], f32, name="inv")
                    nc.vector.reciprocal(inv[:], tp[:, H : H + 1])
                    nc.vector.tensor_scalar_mul(
                        outall[:, tq * 4 + j, :], tp[:, 0:H], inv[:]
                    )
                    # store each 128-query block as soon as it's scaled so
                    # only a 32KB store trails the last attnval group
                    nc.sync.dma_start(
                        out_r[:, tq * 4 + j, :], outall[:, tq * 4 + j, :]
                    )

            def _feed(job):
                """Feed one (tq, tiles) job into the score/exp/attnval
                pipeline; attnval trails by PIPE groups."""
                if job is not None:
                    tq, g = job
                    tsl = slice(tq * 512, (tq + 1) * 512)
                    sp = spp.tile([P, GROUP, 512], f32, name="sp", tag="sp",
                                  bufs=2)
                    for i, mt in enumerate(g):
                        msl = slice(mt * P, (mt + 1) * P)
                        diag = _is_diag(tq, mt)
                        lo = (mt - tq * 4) * P if diag else 0
                        nc.tensor.matmul(
                            sp[:, i, lo:512],
                            K2[:, msl],
                            q2T[:, tq * 512 + lo : (tq + 1) * 512],
                            start=True, stop=not diag,
                        )
                        if diag:
                            # triangle spans only the first 128 live columns
                            nc.tensor.matmul(
                                sp[:, i, lo : lo + P], id_sb[:, :],
                                maskA[:, 0, 0:P],
                                start=False, stop=True,
                            )
                    ex = exp_pool.tile([P, GROUP, 512], bf16, name="ex")
                    # exp only the live column range: for a group of
                    # diagonal tiles, columns left of every tile's triangle
                    # are never read by attnval, and the tail is exp-bound
                    glo = min(
                        (mt - tq * 4) * P if _is_diag(tq, mt) else 0
                        for mt in g
                    )
                    nc.scalar.activation(
                        ex[:, 0 : len(g), glo:512],
                        sp[:, 0 : len(g), glo:512], Exp
                    )
                    pend.append((tq, g, ex))
                if (len(pend) > PIPE) or (job is None and pend):
                    ptq, pg, pex = pend.pop(0)
                    if ptq not in oT_ps:
                        oT_ps[ptq] = accp.tile([H + 1, 512], f32, name="oT_ps",
                                               tag="acc", bufs=2)
                    for i, mt in enumerate(pg):
                        lo = (mt - ptq * 4) * P if _is_diag(ptq, mt) else 0
                        nc.tensor.matmul(
                            oT_ps[ptq][:, lo:512],
                            valaug[:, mt, :], pex[:, i, lo:512],
                            start=(n_done[ptq] == 0),
                            stop=(n_done[ptq] == ntiles[ptq] - 1),
                        )
                        n_done[ptq] += 1
                    if n_done[ptq] == ntiles[ptq]:
                        _epilogue(ptq)

            # ---- x/pe stream + projections, attention jobs interleaved --
            for pi, c in enumerate(CHUNK_ORDER):
                sl = slice(c * 512, (c + 1) * 512)
                if pi == 0:
                    xt, pet = xt0, pet0
                else:
                    xt = xpool.tile([P, DC, 512], f16, name="xt", tag="xt")
                    nc.sync.dma_start(xt[:], xT_r[:, :, sl])
                    pet = None
                    if c not in GCH:
                        pet = pepool.tile([P, DC, 512], f16, name="pet",
                                          tag="pet")
                        nc.sync.dma_start(pet[:], peT_r[:, :, sl])

                kvp = ppj.tile([P, 512], f32, name="kvp", tag="proj", bufs=2)
                for dc in range(DC):
                    nc.tensor.matmul(
                        kvp[:], wkv_sb[:, dc, :], xt[:, dc, :],
                        start=(dc == 0), stop=(dc == DC - 1),
                    )
                nc.vector.tensor_scalar_add(
                    K2[0:H, sl], kvp[0:H, :], bkv_sb[0:H, :]
                )
                vtmp = vtmp_pool.tile([H, 512], f16, name="vtmp", tag="vtmp")
                nc.vector.tensor_scalar_add(
                    vtmp[:], kvp[H:P, :], bkv_sb[H : 2 * H, :]
                )

                if pi == 0:
                    # r shard for the gathered chunks: this core projects its
                    # 192 posenc columns; one AllGather (~20us, on the
                    # collective cores) lands them by ~35us, well before the
                    # first reader (pair-5 key bias at ~35us)
                    rps = ppj.tile([P, GSH], f32, name="rps", tag="proj",
                                   bufs=2)
                    for dc in range(DC):
                        nc.tensor.matmul(
                            rps[:], wr2_sb[:, dc, :], peS_sb[:, dc, :],
                            start=(dc == 0), stop=(dc == DC - 1),
                        )
                    rsh_sb = vtmp_pool.tile([P, GSH], f16, name="rsh_sb",
                                            tag="rsh")
                    nc.vector.tensor_scalar_add(
                        rsh_sb[H:P, :], rps[H:P, :], brc_sb[H : 2 * H, :]
                    )
                    rloc_dr = dramp.tile([H, GSH], f16, name="rloc_dr")
                    nc.scalar.dma_start(rloc_dr[:], rsh_sb[H:P, :])
                    rTg_dr = dramp.tile([NCORES * H, GSH], f16, name="rTg_dr",
                                        addr_space="Shared")
                    nc.gpsimd.collective_compute(
                        "AllGather", mybir.AluOpType.bypass,
                        replica_groups=[list(range(NCORES))],
                        ins=[rloc_dr[:]], outs=[rTg_dr[:]],
                    )
                    # on the Pool queue: nothing behind it, so its SEQ wait
                    # on the collective can't head-of-line-block anything
                    nc.gpsimd.dma_start(
                        K2[H:P, 512:2048].rearrange("h (b m) -> h b m",
                                                    b=NCORES),
                        rTg_dr[:].rearrange("(b h) m -> h b m", h=H),
                    )

                if c < NTQ:
                    qp = ppj.tile([P, 512], f32, name="qp", tag="proj", bufs=2)
                    for dc in range(DC):
                        nc.tensor.matmul(
                            qp[:], wqq_sb[:, dc, :], xt[:, dc, :],
                            start=(dc == 0), stop=(dc == DC - 1),
                        )
                    nc.vector.tensor_scalar_add(q2T[:, sl], qp[:], bqq_sb[:])

                # r projection for this chunk (rows 64:128 of wr2 = Wr);
                # chunks in GCH get their r rows from the AllGather instead
                if c not in GCH:
                    rp = ppj.tile([P, 512], f32, name="rp", tag="proj", bufs=2)
                    for dc in range(DC):
                        nc.tensor.matmul(
                            rp[:], wr2_sb[:, dc, :], pet[:, dc, :],
                            start=(dc == 0), stop=(dc == DC - 1),
                        )
                    nc.vector.tensor_scalar_add(
                        K2[H:P, sl], rp[H:P, :], brc_sb[H : 2 * H, :]
                    )

                # transpose v chunk -> valaug rows (4 tiles batched per bank)
                vps = ppj.tile([P, 4, H], f16, name="vps", tag="proj", bufs=2)
                for j in range(4):
                    nc.tensor.transpose(
                        vps[:, j, :], vtmp[:, j * P : (j + 1) * P],
                        id_sb[0:H, 0:H],
                    )
                nc.vector.tensor_copy(valaug[:, 4 * c : 4 * c + 4, 0:H], vps[:])

                # key bias -> eb, folded into valaug (needs K2 both halves)
                bp = ppj.tile([P, 16], f32, name="bp", tag="proj", bufs=2)
                for j in range(4):
                    mt = c * 4 + j
                    msl = slice(mt * P, (mt + 1) * P)
                    nc.tensor.matmul(
                        bp[:, 4 * j : 4 * j + 4], K2[:, msl], uv_col[:],
                        start=True, stop=True,
                    )
                nc.scalar.activation(
                    ebias[:, c * 4 : (c + 1) * 4], bp[:, 0:16:4], Exp
                )
                for j in range(4):
                    mt = c * 4 + j
                    nc.vector.tensor_scalar_mul(
                        valaug[:, mt, :], valaug[:, mt, :],
                        ebias[:, mt : mt + 1],
                    )

                # feed attention jobs that this pair unblocked
                for job in jobs_at[pi]:
                    _feed(job)

            # drain the attnval pipeline
            for _ in range(PIPE):
                _feed(None)

    nc.compile()
    return nc


def _get_module():
    if "nc" not in _CACHE:
        _CACHE["nc"] = _build_module()
    return _CACHE["nc"]


def _pack_pc(w):
    """[D, W] -> [128, DC*W] SBUF image: row p = concat_c w[c*128+p, :]."""
    wid = w.shape[1]
    return np.ascontiguousarray(
        w.reshape(DC, P, wid).transpose(1, 0, 2).reshape(P, DC * wid)
    )


def _make_in_maps(inputs):
    inp = np.asarray(inputs["inp_data"], dtype=np.float32)
    Wq = np.asarray(inputs["Wq"], dtype=np.float32)
    bq = np.asarray(inputs["bq"], dtype=np.float32)
    Wk = np.asarray(inputs["Wk"], dtype=np.float32)
    bk = np.asarray(inputs["bk"], dtype=np.float32)
    Wv = np.asarray(inputs["Wv"], dtype=np.float32)
    bv = np.asarray(inputs["bv"], dtype=np.float32)
    Wr = np.asarray(inputs["Wr"], dtype=np.float32)
    br = np.asarray(inputs["br"], dtype=np.float32)
    u = np.asarray(inputs["u"], dtype=np.float32)
    v = np.asarray(inputs["v"], dtype=np.float32)

    if "peT16" not in _CACHE:
        _CACHE["peT16"] = np.ascontiguousarray(_posenc_T().astype(np.float16))
    f16 = np.float16
    common = {
        "peT": _CACHE["peT16"],
        "wkv": _pack_pc(np.concatenate([Wk, Wv], axis=1).astype(f16)),
        "wqq": _pack_pc(np.concatenate([Wq, Wq], axis=1).astype(f16)),
        "wr2": _pack_pc(
            np.concatenate([np.zeros_like(Wr), Wr], axis=1).astype(f16)
        ),
        "bkv": np.ascontiguousarray(np.concatenate([bk, bv]).reshape(2 * H, 1)),
        "bqq": np.ascontiguousarray(np.concatenate([bq, bq]).reshape(2 * H, 1)),
        "br": np.ascontiguousarray(br.reshape(H, 1)),
        "u": np.ascontiguousarray(u.reshape(H, 1)),
        "v": np.ascontiguousarray(v.reshape(H, 1)),
    }
    peT16 = _CACHE["peT16"]
    in_maps = []
    for b in range(NCORES):
        peS = peT16[:, 512 + b * GSH : 512 + (b + 1) * GSH]
        m = {
            "xT": np.ascontiguousarray(inp[:, b, :].T, dtype=f16),
            "peS": _pack_pc(peS),
        }
        m.update(common)
        in_maps.append(m)
    return in_maps


def _run(in_maps, trace=False):
    from concourse.bass_utils import run_bass_kernel_spmd

    nc = _get_module()
    return run_bass_kernel_spmd(
        nc, in_maps, core_ids=list(range(NCORES)), trace=trace
    )


def kernel(**inputs) -> np.ndarray:
    in_maps = _make_in_maps(inputs)
    res = _run(in_maps, trace=False)
    out = np.stack([res.results[b]["out"] for b in range(NCORES)], axis=1)
    return np.ascontiguousarray(out.astype(np.float32))
